# revision 22
# baseline (speedup 1.0000x reference)
"""Trainium2 Bass kernel for nn_PolicyNet4 (topk_masking), 8-core SPMD.

Sharding: data-parallel attention over batch (32 batches/core), then
AllToAll re-shard of the flattened activation x [256, 100201] so each core
holds a contraction-chunk (13 L-positions x 1000 feats) of x for ALL 256
batches; fc3 contraction-sharded against the matching 1/8 slice of w3
(820MB GEMM split), partial y3 AllReduce'd, then fc5/fc6/topk/softmax
replicated on every core.

Precision: top-50 selection flips are the dominant error source, so the
dominant paths (embed, fc3, fc5, fc6) run fp16-split (hi/lo, 3 products,
~2^-22 operand error) and the attention chain runs plain fp16 (full PE
rate, tf32-grade mantissa). Low parts of small-magnitude weights are
pre-scaled by 2^6 (paired with a 2^-6 scale on the other operand) to stay
out of fp16-subnormal range.
"""

import numpy as np
from contextlib import ExitStack

import concourse.bass as bass
import concourse.mybir as mybir
import concourse.tile as tile
from concourse import bacc
from concourse.bass_utils import run_bass_kernel_spmd
from concourse.masks import make_identity

F32 = mybir.dt.float32
F16 = mybir.dt.float16

B, L, E, H, A, HID = 256, 100, 1000, 10, 101, 2048
NW = 50
DH = E // H
NCORE = 8
BC = B // NCORE            # 32 batches per core
TOK = BC * L               # 3200 tokens per core
ET = 125                   # e-tile (E = 8*125)
NE = E // ET               # 8
GSZ = 400                  # tokens per group
NBG = GSZ // L             # 4 batches per group
NG = TOK // GSZ            # 8 groups
FLATP = L * ET             # per-core flat chunk: e-slice for all l (12500)
KT = L                     # 100 k-tiles of 125 (one per l)
NQK = 2 * E // DH          # 20 qk head-tiles of 100
LO_SCALE = 6               # lo parts pre-scaled by 2**LO_SCALE
LSF = float(2.0 ** LO_SCALE)
LSI = float(2.0 ** (-LO_SCALE))

_CACHED = None


def _split(x):
    """Weight fp16 split: hi = fp16(x), lo = fp16((x - hi) * 2**LO_SCALE).
    The 2**LO_SCALE keeps tiny weight-lo values out of fp16-subnormal range;
    it is compensated by pairing lo with a 2**-LO_SCALE-scaled activation."""
    x = np.ascontiguousarray(x, np.float32)
    hi = x.astype(np.float16)
    lo = ((x - hi.astype(np.float32)) * LSF).astype(np.float16)
    return hi, lo


def _split0(x):
    """Activation fp16 split: hi = fp16(x), lo = fp16(x - hi), unscaled
    (paired directly with weight-hi)."""
    x = np.ascontiguousarray(x, np.float32)
    hi = x.astype(np.float16)
    lo = (x - hi.astype(np.float32)).astype(np.float16)
    return hi, lo


def _build():
    nc = bacc.Bacc("TRN2", target_bir_lowering=False, debug=False,
                   num_devices=NCORE)

    def inp(name, shape, dt=F16):
        return nc.declare_dram_parameter(name, list(shape), dt, isOutput=False)

    xh_p = inp("xh", [E, TOK])
    xl_p = inp("xl", [E, TOK])
    weh_p = inp("weh", [E, E])
    wel_p = inp("wel", [E, E])
    be_p = inp("be", [E], F32)
    win_p = inp("win", [E, 3 * E])
    wout_p = inp("wout", [E, E])
    w3h_p = inp("w3h", [FLATP, HID])
    w3l_p = inp("w3l", [FLATP, HID])
    b3_p = inp("b3", [HID], F32)
    lah_p = inp("lah", [A, B])
    lal_p = inp("lal", [A, B])
    w3lah_p = inp("w3lah", [A, HID])
    w3lal_p = inp("w3lal", [A, HID])
    w5h_p = inp("w5h", [HID, A])
    w5l_p = inp("w5l", [HID, A])
    b5_p = inp("b5", [A], F32)
    w6ah_p = inp("w6ah", [A, A])
    w6al_p = inp("w6al", [A, A])
    w6bh_p = inp("w6bh", [A, A])
    w6bl_p = inp("w6bl", [A, A])
    b6_p = inp("b6", [A], F32)
    out_p = nc.declare_dram_parameter("out", [B, A], F32, isOutput=True)

    EXP = mybir.ActivationFunctionType.Exp
    RELU = mybir.ActivationFunctionType.Relu
    IDENT = mybir.ActivationFunctionType.Identity
    MULT = mybir.AluOpType.mult
    ADD = mybir.AluOpType.add

    with tile.TileContext(nc) as tc:
        with tc.tile_pool(name="const", bufs=1) as const, \
             tc.tile_pool(name="dram", bufs=1, space="DRAM") as dram:

            stkW = ExitStack()
            beTp = stkW.enter_context(tc.tile_pool(name="beTp", bufs=NE))
            wres8 = stkW.enter_context(tc.tile_pool(name="wres8", bufs=NE))
            wresA = stkW.enter_context(tc.tile_pool(name="wresA", bufs=H))
            id16 = const.tile([128, 128], F16)
            make_identity(nc, id16[:])
            id32 = const.tile([128, 128], F32)
            make_identity(nc, id32[:])

            beT = []
            for k in range(NE):
                t = beTp.tile([ET, 1], F32, tag="beT")
                nc.sync.dma_start(t[:], be_p[k * ET:(k + 1) * ET])
                beT.append(t)

            # resident embed weights (split)
            weh_t, wel_t, win_t, wout_t = [], [], [], []
            for k in range(NE):
                t = wres8.tile([ET, E], F16, tag="weh")
                nc.sync.dma_start(t[:], weh_p[k * ET:(k + 1) * ET, :])
                weh_t.append(t)
                t = wres8.tile([ET, E], F16, tag="wel")
                nc.sync.dma_start(t[:], wel_p[k * ET:(k + 1) * ET, :])
                wel_t.append(t)
                t = wres8.tile([ET, 3 * E], F16, tag="win")
                nc.sync.dma_start(t[:], win_p[k * ET:(k + 1) * ET, :])
                win_t.append(t)
            for kk in range(H):
                t = wresA.tile([DH, E], F16, tag="wout")
                nc.sync.dma_start(t[:], wout_p[kk * DH:(kk + 1) * DH, :])
                wout_t.append(t)

            # collective DRAM buffers
            a2a_in = dram.tile([NCORE, 2, ET, TOK], F16)
            a2a_out = dram.tile([NCORE, 2, ET, TOK], F16)
            ar_in = dram.tile([B, HID], F32)
            ar_out = dram.tile([B, HID], F32, addr_space="Shared")


            # ================= PHASE A: attention =================
            stkA = ExitStack()
            inx = stkA.enter_context(tc.tile_pool(name="inx", bufs=9))
            h32p = stkA.enter_context(tc.tile_pool(name="h32", bufs=9))
            h16p = stkA.enter_context(tc.tile_pool(name="h16", bufs=9))
            qkp = stkA.enter_context(tc.tile_pool(name="qk", bufs=21))
            vvp = stkA.enter_context(tc.tile_pool(name="vv", bufs=5))
            oTp = stkA.enter_context(tc.tile_pool(name="oT", bufs=11))
            x32p = stkA.enter_context(tc.tile_pool(name="x32", bufs=3))
            xspp = stkA.enter_context(tc.tile_pool(name="xsp", bufs=3))
            atp = stkA.enter_context(tc.tile_pool(name="at", bufs=8))
            psb = stkA.enter_context(tc.tile_pool(name="ps_big", bufs=3, space="PSUM"))
            psa = stkA.enter_context(tc.tile_pool(name="ps_at", bufs=3, space="PSUM"))
            pstr = stkA.enter_context(tc.tile_pool(name="ps_tr", bufs=2, space="PSUM"))
            if True:

                for g in range(NG):
                    t0 = g * GSZ
                    # ---- load x tiles (hi/lo) for this group ----
                    xh_in, xl_in, xhs_in = [], [], []
                    for k in range(NE):
                        t = inx.tile([ET, GSZ], F16, tag="xh_in")
                        nc.sync.dma_start(t[:], xh_p[k * ET:(k + 1) * ET, t0:t0 + GSZ])
                        xh_in.append(t)
                        t = inx.tile([ET, GSZ], F16, tag="xl_in")
                        nc.sync.dma_start(t[:], xl_p[k * ET:(k + 1) * ET, t0:t0 + GSZ])
                        xl_in.append(t)
                        t = inx.tile([ET, GSZ], F16, tag="xhs_in")
                        nc.vector.tensor_scalar_mul(t[:], xh_in[k][:], LSI)
                        xhs_in.append(t)
                    # ---- mm1: hT = We @ xT (+be), fp16-split ----
                    h32_t, h16_t = [], []
                    for m in range(NE):
                        ms = slice(m * ET, (m + 1) * ET)
                        ps = psb.tile([ET, GSZ], F32, tag="psb")
                        for k in range(NE):
                            nc.tensor.matmul(ps[:], weh_t[k][:, ms], xh_in[k][:],
                                             start=(k == 0), stop=False)
                            nc.tensor.matmul(ps[:], weh_t[k][:, ms], xl_in[k][:],
                                             start=False, stop=False)
                            nc.tensor.matmul(ps[:], wel_t[k][:, ms], xhs_in[k][:],
                                             start=False, stop=(k == NE - 1))
                        h32 = h32p.tile([ET, GSZ], F32, tag="h32")
                        nc.scalar.activation(h32[:], ps[:], IDENT, bias=beT[m][:])
                        h32_t.append(h32)
                        h16 = h16p.tile([ET, GSZ], F16, tag="h16")
                        nc.vector.tensor_copy(h16[:], h32[:])
                        h16_t.append(h16)
                    # ---- mm2qk: qkT[feat, tok] fp16 ----
                    qk_t = []
                    for mq in range(NQK):
                        ms = slice(mq * DH, (mq + 1) * DH)
                        ps = psb.tile([DH, GSZ], F32, tag="psb")
                        for k in range(NE):
                            nc.tensor.matmul(ps[:], win_t[k][:, ms], h16_t[k][:],
                                             start=(k == 0), stop=(k == NE - 1))
                        qk = qkp.tile([DH, GSZ], F16, tag="qk")
                        nc.vector.tensor_copy(qk[:], ps[:])
                        qk_t.append(qk)
                    # ---- mm2v: v[tok, feat] fp16, per batch ----
                    v_t = []
                    for bt in range(NBG):
                        bs = slice(bt * L, (bt + 1) * L)
                        v = vvp.tile([L, E], F16, tag="v")
                        for n in range(2):
                            ns = slice(2 * E + n * 500, 2 * E + (n + 1) * 500)
                            ps = psb.tile([L, 500], F32, tag="psb")
                            for k in range(NE):
                                nc.tensor.matmul(ps[:], h16_t[k][:, bs],
                                                 win_t[k][:, ns],
                                                 start=(k == 0), stop=(k == NE - 1))
                            nc.vector.tensor_copy(v[:, n * 500:(n + 1) * 500], ps[:])
                        v_t.append(v)
                    # ---- attention per (batch, head) ----
                    oT_t = [oTp.tile([DH, GSZ], F16, name=f"oT{_h}", tag="oT") for _h in range(H)]
                    for bt in range(NBG):
                        bs = slice(bt * L, (bt + 1) * L)
                        for h in range(H):
                            s_ps = psa.tile([L, L], F32, tag="psa")
                            nc.tensor.matmul(s_ps[:], qk_t[h][:, bs],
                                             qk_t[H + h][:, bs],
                                             start=True, stop=True)
                            e_t = atp.tile([L, L], F32, tag="e")
                            rs = atp.tile([L, 1], F32, tag="rs")
                            nc.scalar.activation(e_t[:], s_ps[:], EXP,
                                                 scale=0.1, accum_out=rs[:])
                            rc = atp.tile([L, 1], F32, tag="rc")
                            nc.vector.reciprocal(rc[:], rs[:])
                            a16 = atp.tile([L, L], F16, tag="a16")
                            nc.vector.tensor_tensor(
                                a16[:], e_t[:], rc[:].to_broadcast([L, L]), MULT)
                            tr_ps = pstr.tile([L, L], F16, tag="psa_t")
                            nc.tensor.transpose(tr_ps[:], a16[:], id16[:L, :L])
                            aT = atp.tile([L, L], F16, tag="aT")
                            nc.vector.tensor_copy(aT[:], tr_ps[:])
                            u_ps = psa.tile([L, L], F32, tag="psa")
                            nc.tensor.matmul(u_ps[:],
                                             v_t[bt][:, h * DH:(h + 1) * DH],
                                             aT[:], start=True, stop=True)
                            nc.vector.tensor_copy(oT_t[h][:, bs], u_ps[:])
                    # ---- mm4 + residual + split + shard write ----
                    for m in range(NE):
                        ms = slice(m * ET, (m + 1) * ET)
                        ps = psb.tile([ET, GSZ], F32, tag="psb")
                        for kk in range(H):
                            nc.tensor.matmul(ps[:], wout_t[kk][:, ms], oT_t[kk][:],
                                             start=(kk == 0), stop=(kk == H - 1))
                        x32 = x32p.tile([ET, GSZ], F32, tag="x32")
                        nc.vector.tensor_add(x32[:], ps[:], h32_t[m][:])
                        xh_m = xspp.tile([ET, GSZ], F16, tag="xh_m")
                        nc.vector.tensor_copy(xh_m[:], x32[:])
                        xl32 = x32p.tile([ET, GSZ], F32, tag="xl32")
                        nc.vector.tensor_sub(xl32[:], x32[:], xh_m[:])
                        xl_m = xspp.tile([ET, GSZ], F16, tag="xl_m")
                        nc.vector.tensor_copy(xl_m[:], xl32[:])
                        nc.sync.dma_start(a2a_in[m, 0, :, t0:t0 + GSZ], xh_m[:])
                        nc.sync.dma_start(a2a_in[m, 1, :, t0:t0 + GSZ], xl_m[:])

            stkA.close()
            stkW.close()
            # ================= A2A =================
            nc.gpsimd.collective_compute(
                "AllToAll", mybir.AluOpType.bypass,
                replica_groups=[list(range(NCORE))],
                ins=[a2a_in.opt()], outs=[a2a_out.opt()])

            # ================= fc3 =================
            stk3 = ExitStack()
            xcp = stk3.enter_context(tc.tile_pool(name="xc", bufs=3))
            w3tp = stk3.enter_context(tc.tile_pool(name="w3t", bufs=4))
            la3p = stk3.enter_context(tc.tile_pool(name="la3", bufs=3))
            y3ep = stk3.enter_context(tc.tile_pool(name="y3e", bufs=2))
            ps3 = stk3.enter_context(tc.tile_pool(name="ps3", bufs=8, space="PSUM"))
            if True:
                xc_h = xcp.tile([ET, NCORE, BC, L], F16, tag="xc_big")
                xc_l = xcp.tile([ET, NCORE, BC, L], F16, tag="xc_big")
                for j in range(NCORE):
                    nc.sync.dma_start(xc_h[:, j, :, :], a2a_out[j, 0, :, :])
                    nc.sync.dma_start(xc_l[:, j, :, :], a2a_out[j, 1, :, :])
                xc_s = xcp.tile([ET, NCORE, BC, L], F16, tag="xc_big")
                nc.vector.tensor_scalar_mul(xc_s[:, :, :, :], xc_h[:, :, :, :], LSI)

                psY = [ps3.tile([128, 512], F32, name=f"psY{_i}", tag="psY") for _i in range(8)]

                # last_action contribution (w3la pre-divided by 8 host-side)
                lah_t = la3p.tile([A, B], F16, tag="la")
                nc.sync.dma_start(lah_t[:], lah_p[:, :])
                lal_t = la3p.tile([A, B], F16, tag="la")
                nc.sync.dma_start(lal_t[:], lal_p[:, :])
                lahs_t = la3p.tile([A, B], F16, tag="la")
                nc.vector.tensor_scalar_mul(lahs_t[:], lah_t[:], LSI)
                w3lah_t = la3p.tile([A, HID], F16, tag="w3la")
                nc.sync.dma_start(w3lah_t[:], w3lah_p[:, :])
                w3lal_t = la3p.tile([A, HID], F16, tag="w3la")
                nc.sync.dma_start(w3lal_t[:], w3lal_p[:, :])
                for half in range(2):
                    hs = slice(half * 128, (half + 1) * 128)
                    for n in range(4):
                        ns = slice(n * 512, (n + 1) * 512)
                        p = psY[half * 4 + n]
                        nc.tensor.matmul(p[:], lah_t[:, hs], w3lah_t[:, ns],
                                         start=True, stop=False)
                        nc.tensor.matmul(p[:], lal_t[:, hs], w3lah_t[:, ns],
                                         start=False, stop=False)
                        nc.tensor.matmul(p[:], lahs_t[:, hs], w3lal_t[:, ns],
                                         start=False, stop=False)

                for kt in range(KT):
                    w3h_t = w3tp.tile([ET, HID], F16, tag="w3t")
                    nc.sync.dma_start(w3h_t[:], w3h_p[kt * ET:(kt + 1) * ET, :])
                    w3l_t = w3tp.tile([ET, HID], F16, tag="w3t")
                    nc.sync.dma_start(w3l_t[:], w3l_p[kt * ET:(kt + 1) * ET, :])
                    for half in range(2):
                        js = slice(half * 4, (half + 1) * 4)
                        lh = xc_h[:, js, :, kt]
                        ll = xc_l[:, js, :, kt]
                        l6 = xc_s[:, js, :, kt]
                        last = (kt == KT - 1)
                        for n in range(4):
                            ns = slice(n * 512, (n + 1) * 512)
                            p = psY[half * 4 + n]
                            nc.tensor.matmul(p[:], lh, w3h_t[:, ns],
                                             start=False, stop=False)
                            nc.tensor.matmul(p[:], ll, w3h_t[:, ns],
                                             start=False, stop=False)
                            nc.tensor.matmul(p[:], l6, w3l_t[:, ns],
                                             start=False, stop=last)
                for half in range(2):
                    y3e = y3ep.tile([128, HID], F32, tag="y3e")
                    for n in range(4):
                        nc.vector.tensor_copy(
                            y3e[:, n * 512:(n + 1) * 512], psY[half * 4 + n][:])
                    nc.sync.dma_start(ar_in[half * 128:(half + 1) * 128, :],
                                      y3e[:])

            stk3.close()
            nc.gpsimd.collective_compute(
                "AllReduce", mybir.AluOpType.add,
                replica_groups=[list(range(NCORE))],
                ins=[ar_in.opt()], outs=[ar_out.opt()])

            # ================= finale (replicated) =================
            arT = ar_out[:, :].rearrange("b h -> h b")
            NHT = HID // 128  # 16
            stkF = ExitStack()
            fin = stkF.enter_context(tc.tile_pool(name="fin", bufs=1))
            finy = stkF.enter_context(tc.tile_pool(name="finy", bufs=NHT))
            fin2 = stkF.enter_context(tc.tile_pool(name="fin2", bufs=4))
            psf = stkF.enter_context(tc.tile_pool(name="psf", bufs=4, space="PSUM"))
            if True:
                y3h_t, y3l_t, y3s_t, w5h_t, w5l_t = [], [], [], [], []
                for ht in range(NHT):
                    hs = slice(ht * 128, (ht + 1) * 128)
                    b3T = fin2.tile([128, 1], F32, tag="b3T")
                    nc.sync.dma_start(b3T[:], b3_p[hs])
                    yt = fin2.tile([128, B], F32, tag="yt")
                    nc.sync.dma_start(yt[:], arT[hs, :])
                    yr = fin2.tile([128, B], F32, tag="yr")
                    nc.scalar.activation(yr[:], yt[:], RELU, bias=b3T[:])
                    yh = finy.tile([128, B], F16, tag="y3h")
                    nc.vector.tensor_copy(yh[:], yr[:])
                    y3h_t.append(yh)
                    yl32 = fin2.tile([128, B], F32, tag="yl32")
                    nc.vector.tensor_sub(yl32[:], yr[:], yh[:])
                    yl = finy.tile([128, B], F16, tag="y3l")
                    nc.vector.tensor_copy(yl[:], yl32[:])
                    y3l_t.append(yl)
                    ys = finy.tile([128, B], F16, tag="y3s")
                    nc.vector.tensor_scalar_mul(ys[:], yh[:], LSI)
                    y3s_t.append(ys)
                    t = finy.tile([128, A], F16, tag="w5h")
                    nc.sync.dma_start(t[:], w5h_p[hs, :])
                    w5h_t.append(t)
                    t = finy.tile([128, A], F16, tag="w5l")
                    nc.sync.dma_start(t[:], w5l_p[hs, :])
                    w5l_t.append(t)
                ps_z = psf.tile([A, B], F32, tag="psf")
                for ht in range(NHT):
                    nc.tensor.matmul(ps_z[:], w5h_t[ht][:], y3h_t[ht][:],
                                     start=(ht == 0), stop=False)
                    nc.tensor.matmul(ps_z[:], w5h_t[ht][:], y3l_t[ht][:],
                                     start=False, stop=False)
                    nc.tensor.matmul(ps_z[:], w5l_t[ht][:], y3s_t[ht][:],
                                     start=False, stop=(ht == NHT - 1))
                b5T = fin.tile([A, 1], F32, tag="b5T")
                nc.sync.dma_start(b5T[:], b5_p[:])
                zT = fin.tile([A, B], F32, tag="zT")
                nc.scalar.activation(zT[:], ps_z[:], IDENT, bias=b5T[:])
                zh = fin.tile([A, B], F16, tag="zh")
                nc.vector.tensor_copy(zh[:], zT[:])
                zl32 = fin.tile([A, B], F32, tag="zl32")
                nc.vector.tensor_sub(zl32[:], zT[:], zh[:])
                zl = fin.tile([A, B], F16, tag="zl")
                nc.vector.tensor_copy(zl[:], zl32[:])
                zs = fin.tile([A, B], F16, tag="zs")
                nc.vector.tensor_scalar_mul(zs[:], zh[:], LSI)
                la2h = fin.tile([A, B], F16, tag="la2h")
                nc.sync.dma_start(la2h[:], lah_p[:, :])
                la2l = fin.tile([A, B], F16, tag="la2l")
                nc.sync.dma_start(la2l[:], lal_p[:, :])
                la2s = fin.tile([A, B], F16, tag="la2s")
                nc.vector.tensor_scalar_mul(la2s[:], la2h[:], LSI)
                w6 = {}
                for nm, p in (("ah", w6ah_p), ("al", w6al_p),
                              ("bh", w6bh_p), ("bl", w6bl_p)):
                    t = fin.tile([A, A], F16, tag="w6" + nm)
                    nc.sync.dma_start(t[:], p[:, :])
                    w6[nm] = t
                ps_f = psf.tile([A, B], F32, tag="psf")
                nc.tensor.matmul(ps_f[:], w6["ah"][:], zh[:], start=True, stop=False)
                nc.tensor.matmul(ps_f[:], w6["ah"][:], zl[:], start=False, stop=False)
                nc.tensor.matmul(ps_f[:], w6["al"][:], zs[:], start=False, stop=False)
                nc.tensor.matmul(ps_f[:], w6["bh"][:], la2h[:], start=False, stop=False)
                nc.tensor.matmul(ps_f[:], w6["bh"][:], la2l[:], start=False, stop=False)
                nc.tensor.matmul(ps_f[:], w6["bl"][:], la2s[:], start=False, stop=True)
                b6T = fin.tile([A, 1], F32, tag="b6T")
                nc.sync.dma_start(b6T[:], b6_p[:])
                fsT = fin.tile([A, B], F32, tag="fsT")
                nc.scalar.activation(fsT[:], ps_f[:], IDENT, bias=b6T[:])

                for half in range(2):
                    hs = slice(half * 128, (half + 1) * 128)
                    tr_ps = psf.tile([128, A], F32, tag="psf")
                    nc.tensor.transpose(tr_ps[:], fsT[:, hs], id32[:A, :A])
                    fs = fin2.tile([128, A], F32, tag="fs")
                    nc.vector.tensor_copy(fs[:], tr_ps[:])
                    # top-50 mask via iterated max8 + match_replace
                    work = fin2.tile([128, A], F32, tag="work")
                    mx8 = fin2.tile([128, 8], F32, tag="mx8")
                    src = fs
                    for k_on in range(0, NW, 8):
                        kthis = min(8, NW - k_on)
                        nc.vector.max(mx8[:], src[:])
                        if kthis < 8:
                            nc.vector.memset(mx8[:, kthis:], -1e30)
                        nc.vector.match_replace(work[:], in_to_replace=mx8[:],
                                                in_values=src[:],
                                                imm_value=-1e30)
                        src = work
                    mask = fin2.tile([128, A], F32, tag="mask")
                    nc.vector.tensor_sub(mask[:], fs[:], work[:])
                    nc.vector.tensor_scalar_min(mask[:], mask[:], 1.0)
                    # masked scores = fs + (mask - 1) * 1e9, winner-exact
                    mt = fin2.tile([128, A], F32, tag="mt")
                    nc.vector.tensor_scalar(mt[:], mask[:], -1.0, 1e9,
                                            mybir.AluOpType.add, MULT)
                    ms = fin2.tile([128, A], F32, tag="ms")
                    nc.vector.tensor_add(ms[:], fs[:], mt[:])
                    nc.vector.max(mx8[:], ms[:])
                    ngm = fin2.tile([128, 1], F32, tag="ngm")
                    nc.vector.tensor_scalar_mul(ngm[:], mx8[:, 0:1], -1.0)
                    e_f = fin2.tile([128, A], F32, tag="e_f")
                    sm = fin2.tile([128, 1], F32, tag="sm")
                    nc.scalar.activation(e_f[:], ms[:], EXP, bias=ngm[:],
                                         accum_out=sm[:])
                    rcf = fin2.tile([128, 1], F32, tag="rcf")
                    nc.vector.reciprocal(rcf[:], sm[:])
                    p_t = fin2.tile([128, A], F32, tag="p_t")
                    nc.vector.tensor_tensor(p_t[:], e_f[:],
                                            rcf[:].to_broadcast([128, A]), MULT)
                    nc.sync.dma_start(out_p[hs, :], p_t[:])
            stkF.close()

    nc.compile()
    return nc


def _prep_inputs(inputs):
    hist = np.ascontiguousarray(inputs["hist_state"], np.float32)
    la = np.ascontiguousarray(inputs["last_action"], np.float32)
    w3 = np.ascontiguousarray(inputs["w3"], np.float32)

    weh, wel = _split(inputs["w_embed"].T)
    winh = np.ascontiguousarray(inputs["in_proj_w"].T, np.float32).astype(np.float16)
    wouth = np.ascontiguousarray(inputs["out_proj_w"].T, np.float32).astype(np.float16)
    lah, lal = _split0(la.T)
    w3lah, w3lal = _split(w3[:, L * E:].T / 8.0)
    w5h, w5l = _split(inputs["w5"].T)
    w6T = np.ascontiguousarray(inputs["w6"], np.float32).T
    w6ah, w6al = _split(w6T[:A])
    w6bh, w6bl = _split(w6T[A:])

    common = {
        "weh": weh, "wel": wel,
        "be": np.ascontiguousarray(inputs["b_embed"], np.float32),
        "win": winh, "wout": wouth,
        "b3": np.ascontiguousarray(inputs["b3"], np.float32),
        "lah": lah, "lal": lal,
        "w3lah": w3lah, "w3lal": w3lal,
        "w5h": w5h, "w5l": w5l,
        "b5": np.ascontiguousarray(inputs["b5"], np.float32),
        "w6ah": w6ah, "w6al": w6al, "w6bh": w6bh, "w6bl": w6bl,
        "b6": np.ascontiguousarray(inputs["b6"], np.float32),
    }
    # w3 body reshaped so core i's slice is e-chunk [125i:125(i+1)) for all l,
    # rows ordered (l, e_local) to match the device kt-loop over l.
    w3r = w3[:, :L * E].reshape(HID, L, E)
    in_maps = []
    for i in range(NCORE):
        xT = hist[i * BC:(i + 1) * BC].reshape(TOK, E).T
        xh, xl = _split0(xT)
        w3c = np.ascontiguousarray(
            w3r[:, :, i * ET:(i + 1) * ET].transpose(1, 2, 0)).reshape(FLATP, HID)
        w3h, w3l = _split(w3c)
        m = dict(common)
        m.update({"xh": xh, "xl": xl, "w3h": w3h, "w3l": w3l})
        in_maps.append(m)
    return in_maps


def kernel(**inputs):
    global _CACHED
    if _CACHED is None:
        _CACHED = _build()
    in_maps = _prep_inputs(inputs)
    res = run_bass_kernel_spmd(_CACHED, in_maps, list(range(NCORE)))
    return np.ascontiguousarray(res.results[0]["out"], np.float32)


if __name__ == "__main__":
    rng = np.random.default_rng(0)
    print("building...")
    _build()
    print("built ok")


# revision 23
# speedup vs baseline: 1.0534x; 1.0534x over previous
"""Trainium2 Bass kernel for nn_PolicyNet4 (topk_masking), 8-core SPMD.

Sharding: data-parallel attention over batch (32 batches/core), then
AllToAll re-shard of the flattened activation x [256, 100201] so each core
holds a contraction-chunk (13 L-positions x 1000 feats) of x for ALL 256
batches; fc3 contraction-sharded against the matching 1/8 slice of w3
(820MB GEMM split), partial y3 AllReduce'd, then fc5/fc6/topk/softmax
replicated on every core.

Precision: top-50 selection flips are the dominant error source, so the
dominant paths (embed, fc3, fc5, fc6) run fp16-split (hi/lo, 3 products,
~2^-22 operand error) and the attention chain runs plain fp16 (full PE
rate, tf32-grade mantissa). Low parts of small-magnitude weights are
pre-scaled by 2^6 (paired with a 2^-6 scale on the other operand) to stay
out of fp16-subnormal range.
"""

import numpy as np
from contextlib import ExitStack

import concourse.bass as bass
import concourse.mybir as mybir
import concourse.tile as tile
from concourse import bacc
from concourse.bass_utils import run_bass_kernel_spmd
from concourse.masks import make_identity

F32 = mybir.dt.float32
F16 = mybir.dt.float16

B, L, E, H, A, HID = 256, 100, 1000, 10, 101, 2048
NW = 50
DH = E // H
NCORE = 8
BC = B // NCORE            # 32 batches per core
TOK = BC * L               # 3200 tokens per core
ET = 125                   # e-tile (E = 8*125)
NE = E // ET               # 8
GSZ = 400                  # tokens per group
NBG = GSZ // L             # 4 batches per group
NG = TOK // GSZ            # 8 groups
FLATP = L * ET             # per-core flat chunk: e-slice for all l (12500)
KT = L                     # 100 k-tiles of 125 (one per l)
NQK = 2 * E // DH          # 20 qk head-tiles of 100
LO_SCALE = 6               # lo parts pre-scaled by 2**LO_SCALE
LSF = float(2.0 ** LO_SCALE)
LSI = float(2.0 ** (-LO_SCALE))

_CACHED = None


def _split(x):
    """Weight fp16 split: hi = fp16(x), lo = fp16((x - hi) * 2**LO_SCALE).
    The 2**LO_SCALE keeps tiny weight-lo values out of fp16-subnormal range;
    it is compensated by pairing lo with a 2**-LO_SCALE-scaled activation."""
    x = np.ascontiguousarray(x, np.float32)
    hi = x.astype(np.float16)
    lo = ((x - hi.astype(np.float32)) * LSF).astype(np.float16)
    return hi, lo


def _split0(x):
    """Activation fp16 split: hi = fp16(x), lo = fp16(x - hi), unscaled
    (paired directly with weight-hi)."""
    x = np.ascontiguousarray(x, np.float32)
    hi = x.astype(np.float16)
    lo = (x - hi.astype(np.float32)).astype(np.float16)
    return hi, lo


def _build():
    nc = bacc.Bacc("TRN2", target_bir_lowering=False, debug=False,
                   num_devices=NCORE)

    def inp(name, shape, dt=F16):
        return nc.declare_dram_parameter(name, list(shape), dt, isOutput=False)

    xh_p = inp("xh", [E, TOK])
    xl_p = inp("xl", [E, TOK])
    weh_p = inp("weh", [E, E])
    wel_p = inp("wel", [E, E])
    be_p = inp("be", [E], F32)
    win_p = inp("win", [E, 3 * E])
    wout_p = inp("wout", [E, E])
    w3h_p = inp("w3h", [FLATP, HID])
    w3l_p = inp("w3l", [FLATP, HID])
    b3_p = inp("b3", [HID], F32)
    lah_p = inp("lah", [A, B])
    lal_p = inp("lal", [A, B])
    w3lah_p = inp("w3lah", [A, HID])
    w3lal_p = inp("w3lal", [A, HID])
    w5h_p = inp("w5h", [HID, A])
    w5l_p = inp("w5l", [HID, A])
    b5_p = inp("b5", [A], F32)
    w6ah_p = inp("w6ah", [A, A])
    w6al_p = inp("w6al", [A, A])
    w6bh_p = inp("w6bh", [A, A])
    w6bl_p = inp("w6bl", [A, A])
    b6_p = inp("b6", [A], F32)
    out_p = nc.declare_dram_parameter("out", [B, A], F32, isOutput=True)

    EXP = mybir.ActivationFunctionType.Exp
    RELU = mybir.ActivationFunctionType.Relu
    IDENT = mybir.ActivationFunctionType.Identity
    MULT = mybir.AluOpType.mult
    ADD = mybir.AluOpType.add

    with tile.TileContext(nc) as tc:
        with tc.tile_pool(name="const", bufs=1) as const, \
             tc.tile_pool(name="dram", bufs=1, space="DRAM") as dram:

            stkW = ExitStack()
            beTp = stkW.enter_context(tc.tile_pool(name="beTp", bufs=NE))
            wres8 = stkW.enter_context(tc.tile_pool(name="wres8", bufs=NE))
            wresA = stkW.enter_context(tc.tile_pool(name="wresA", bufs=H))
            id16 = const.tile([128, 128], F16)
            make_identity(nc, id16[:])
            id32 = const.tile([128, 128], F32)
            make_identity(nc, id32[:])

            beT = []
            for k in range(NE):
                t = beTp.tile([ET, 1], F32, tag="beT")
                nc.sync.dma_start(t[:], be_p[k * ET:(k + 1) * ET])
                beT.append(t)

            # resident embed weights (split)
            weh_t, wel_t, win_t, wout_t = [], [], [], []
            for k in range(NE):
                t = wres8.tile([ET, E], F16, tag="weh")
                nc.sync.dma_start(t[:], weh_p[k * ET:(k + 1) * ET, :])
                weh_t.append(t)
                t = wres8.tile([ET, E], F16, tag="wel")
                nc.sync.dma_start(t[:], wel_p[k * ET:(k + 1) * ET, :])
                wel_t.append(t)
                t = wres8.tile([ET, 3 * E], F16, tag="win")
                nc.sync.dma_start(t[:], win_p[k * ET:(k + 1) * ET, :])
                win_t.append(t)
            for kk in range(H):
                t = wresA.tile([DH, E], F16, tag="wout")
                nc.sync.dma_start(t[:], wout_p[kk * DH:(kk + 1) * DH, :])
                wout_t.append(t)

            # collective DRAM buffers
            a2a_in1 = dram.tile([NCORE, 2, ET, TOK // 2], F16)
            a2a_out1 = dram.tile([NCORE, 2, ET, TOK // 2], F16)
            a2a_in2 = dram.tile([NCORE, 2, ET, TOK // 2], F16)
            a2a_out2 = dram.tile([NCORE, 2, ET, TOK // 2], F16)
            ar_in = dram.tile([B, HID], F32)
            ar_out = dram.tile([B, HID], F32, addr_space="Shared")


            # ================= PHASE A: attention =================
            stkA = ExitStack()
            inx = stkA.enter_context(tc.tile_pool(name="inx", bufs=9))
            h32p = stkA.enter_context(tc.tile_pool(name="h32", bufs=9))
            h16p = stkA.enter_context(tc.tile_pool(name="h16", bufs=9))
            qkp = stkA.enter_context(tc.tile_pool(name="qk", bufs=21))
            vvp = stkA.enter_context(tc.tile_pool(name="vv", bufs=5))
            oTp = stkA.enter_context(tc.tile_pool(name="oT", bufs=11))
            x32p = stkA.enter_context(tc.tile_pool(name="x32", bufs=3))
            xspp = stkA.enter_context(tc.tile_pool(name="xsp", bufs=3))
            atp = stkA.enter_context(tc.tile_pool(name="at", bufs=8))
            psb = stkA.enter_context(tc.tile_pool(name="ps_big", bufs=3, space="PSUM"))
            psa = stkA.enter_context(tc.tile_pool(name="ps_at", bufs=3, space="PSUM"))
            pstr = stkA.enter_context(tc.tile_pool(name="ps_tr", bufs=2, space="PSUM"))
            if True:

                for g in range(NG):
                    t0 = g * GSZ
                    # ---- load x tiles (hi/lo) for this group ----
                    xh_in, xl_in, xhs_in = [], [], []
                    for k in range(NE):
                        t = inx.tile([ET, GSZ], F16, tag="xh_in")
                        nc.sync.dma_start(t[:], xh_p[k * ET:(k + 1) * ET, t0:t0 + GSZ])
                        xh_in.append(t)
                        t = inx.tile([ET, GSZ], F16, tag="xl_in")
                        nc.sync.dma_start(t[:], xl_p[k * ET:(k + 1) * ET, t0:t0 + GSZ])
                        xl_in.append(t)
                        t = inx.tile([ET, GSZ], F16, tag="xhs_in")
                        nc.vector.tensor_scalar_mul(t[:], xh_in[k][:], LSI)
                        xhs_in.append(t)
                    # ---- mm1: hT = We @ xT (+be), fp16-split ----
                    h32_t, h16_t = [], []
                    for m in range(NE):
                        ms = slice(m * ET, (m + 1) * ET)
                        ps = psb.tile([ET, GSZ], F32, tag="psb")
                        for k in range(NE):
                            nc.tensor.matmul(ps[:], weh_t[k][:, ms], xh_in[k][:],
                                             start=(k == 0), stop=False)
                            nc.tensor.matmul(ps[:], weh_t[k][:, ms], xl_in[k][:],
                                             start=False, stop=False)
                            nc.tensor.matmul(ps[:], wel_t[k][:, ms], xhs_in[k][:],
                                             start=False, stop=(k == NE - 1))
                        h32 = h32p.tile([ET, GSZ], F32, tag="h32")
                        nc.scalar.activation(h32[:], ps[:], IDENT, bias=beT[m][:])
                        h32_t.append(h32)
                        h16 = h16p.tile([ET, GSZ], F16, tag="h16")
                        nc.vector.tensor_copy(h16[:], h32[:])
                        h16_t.append(h16)
                    # ---- mm2qk: qkT[feat, tok] fp16 ----
                    qk_t = []
                    for mq in range(NQK):
                        ms = slice(mq * DH, (mq + 1) * DH)
                        ps = psb.tile([DH, GSZ], F32, tag="psb")
                        for k in range(NE):
                            nc.tensor.matmul(ps[:], win_t[k][:, ms], h16_t[k][:],
                                             start=(k == 0), stop=(k == NE - 1))
                        qk = qkp.tile([DH, GSZ], F16, tag="qk")
                        nc.vector.tensor_copy(qk[:], ps[:])
                        qk_t.append(qk)
                    # ---- mm2v: v[tok, feat] fp16, per batch ----
                    v_t = []
                    for bt in range(NBG):
                        bs = slice(bt * L, (bt + 1) * L)
                        v = vvp.tile([L, E], F16, tag="v")
                        for n in range(2):
                            ns = slice(2 * E + n * 500, 2 * E + (n + 1) * 500)
                            ps = psb.tile([L, 500], F32, tag="psb")
                            for k in range(NE):
                                nc.tensor.matmul(ps[:], h16_t[k][:, bs],
                                                 win_t[k][:, ns],
                                                 start=(k == 0), stop=(k == NE - 1))
                            nc.vector.tensor_copy(v[:, n * 500:(n + 1) * 500], ps[:])
                        v_t.append(v)
                    # ---- attention per (batch, head) ----
                    oT_t = [oTp.tile([DH, GSZ], F16, name=f"oT{_h}", tag="oT") for _h in range(H)]
                    for bt in range(NBG):
                        bs = slice(bt * L, (bt + 1) * L)
                        for h in range(H):
                            s_ps = psa.tile([L, L], F32, tag="psa")
                            nc.tensor.matmul(s_ps[:], qk_t[h][:, bs],
                                             qk_t[H + h][:, bs],
                                             start=True, stop=True)
                            e_t = atp.tile([L, L], F32, tag="e")
                            rs = atp.tile([L, 1], F32, tag="rs")
                            nc.scalar.activation(e_t[:], s_ps[:], EXP,
                                                 scale=0.1, accum_out=rs[:])
                            rc = atp.tile([L, 1], F32, tag="rc")
                            nc.vector.reciprocal(rc[:], rs[:])
                            a16 = atp.tile([L, L], F16, tag="a16")
                            nc.vector.tensor_tensor(
                                a16[:], e_t[:], rc[:].to_broadcast([L, L]), MULT)
                            tr_ps = pstr.tile([L, L], F16, tag="psa_t")
                            nc.tensor.transpose(tr_ps[:], a16[:], id16[:L, :L])
                            aT = atp.tile([L, L], F16, tag="aT")
                            nc.vector.tensor_copy(aT[:], tr_ps[:])
                            u_ps = psa.tile([L, L], F32, tag="psa")
                            nc.tensor.matmul(u_ps[:],
                                             v_t[bt][:, h * DH:(h + 1) * DH],
                                             aT[:], start=True, stop=True)
                            nc.vector.tensor_copy(oT_t[h][:, bs], u_ps[:])
                    # ---- mm4 + residual + split + shard write ----
                    for m in range(NE):
                        ms = slice(m * ET, (m + 1) * ET)
                        ps = psb.tile([ET, GSZ], F32, tag="psb")
                        for kk in range(H):
                            nc.tensor.matmul(ps[:], wout_t[kk][:, ms], oT_t[kk][:],
                                             start=(kk == 0), stop=(kk == H - 1))
                        x32 = x32p.tile([ET, GSZ], F32, tag="x32")
                        nc.vector.tensor_add(x32[:], ps[:], h32_t[m][:])
                        xh_m = xspp.tile([ET, GSZ], F16, tag="xh_m")
                        nc.vector.tensor_copy(xh_m[:], x32[:])
                        xl32 = x32p.tile([ET, GSZ], F32, tag="xl32")
                        nc.vector.tensor_sub(xl32[:], x32[:], xh_m[:])
                        xl_m = xspp.tile([ET, GSZ], F16, tag="xl_m")
                        nc.vector.tensor_copy(xl_m[:], xl32[:])
                        a2a_in_g = a2a_in1 if g < NG // 2 else a2a_in2
                        th = t0 if g < NG // 2 else t0 - TOK // 2
                        nc.sync.dma_start(a2a_in_g[m, 0, :, th:th + GSZ], xh_m[:])
                        nc.sync.dma_start(a2a_in_g[m, 1, :, th:th + GSZ], xl_m[:])
                    if g == NG // 2 - 1:
                        nc.gpsimd.collective_compute(
                            "AllToAll", mybir.AluOpType.bypass,
                            replica_groups=[list(range(NCORE))],
                            ins=[a2a_in1.opt()], outs=[a2a_out1.opt()])

            stkA.close()
            stkW.close()
            # ================= A2A (second half; first fired mid-phase-A) ====
            nc.gpsimd.collective_compute(
                "AllToAll", mybir.AluOpType.bypass,
                replica_groups=[list(range(NCORE))],
                ins=[a2a_in2.opt()], outs=[a2a_out2.opt()])

            # ================= fc3 =================
            stk3 = ExitStack()
            xcp = stk3.enter_context(tc.tile_pool(name="xc", bufs=2))
            w3tp = stk3.enter_context(tc.tile_pool(name="w3t", bufs=8))
            xssp = stk3.enter_context(tc.tile_pool(name="xss", bufs=4))
            la3p = stk3.enter_context(tc.tile_pool(name="la3", bufs=3))
            y3ep = stk3.enter_context(tc.tile_pool(name="y3e", bufs=2))
            ps3 = stk3.enter_context(tc.tile_pool(name="ps3", bufs=8, space="PSUM"))
            if True:
                xc_h = xcp.tile([ET, NCORE, BC, L], F16, tag="xc_big")
                xc_l = xcp.tile([ET, NCORE, BC, L], F16, tag="xc_big")
                HB = BC // 2
                for j in range(NCORE):
                    nc.sync.dma_start(xc_h[:, j, :HB, :], a2a_out1[j, 0, :, :])
                    nc.sync.dma_start(xc_l[:, j, :HB, :], a2a_out1[j, 1, :, :])
                    nc.sync.dma_start(xc_h[:, j, HB:, :], a2a_out2[j, 0, :, :])
                    nc.sync.dma_start(xc_l[:, j, HB:, :], a2a_out2[j, 1, :, :])

                psY = [ps3.tile([128, 512], F32, name=f"psY{_i}", tag="psY") for _i in range(8)]

                # last_action contribution (w3la pre-divided by 8 host-side)
                lah_t = la3p.tile([A, B], F16, tag="la")
                nc.sync.dma_start(lah_t[:], lah_p[:, :])
                lal_t = la3p.tile([A, B], F16, tag="la")
                nc.sync.dma_start(lal_t[:], lal_p[:, :])
                lahs_t = la3p.tile([A, B], F16, tag="la")
                nc.vector.tensor_scalar_mul(lahs_t[:], lah_t[:], LSI)
                w3lah_t = la3p.tile([A, HID], F16, tag="w3la")
                nc.sync.dma_start(w3lah_t[:], w3lah_p[:, :])
                w3lal_t = la3p.tile([A, HID], F16, tag="w3la")
                nc.sync.dma_start(w3lal_t[:], w3lal_p[:, :])
                for half in range(2):
                    hs = slice(half * 128, (half + 1) * 128)
                    for n in range(4):
                        ns = slice(n * 512, (n + 1) * 512)
                        p = psY[half * 4 + n]
                        nc.tensor.matmul(p[:], lah_t[:, hs], w3lah_t[:, ns],
                                         start=True, stop=False)
                        nc.tensor.matmul(p[:], lal_t[:, hs], w3lah_t[:, ns],
                                         start=False, stop=False)
                        nc.tensor.matmul(p[:], lahs_t[:, hs], w3lal_t[:, ns],
                                         start=False, stop=False)

                for kt in range(KT):
                    w3h_t = w3tp.tile([ET, HID], F16, tag="w3t")
                    nc.sync.dma_start(w3h_t[:], w3h_p[kt * ET:(kt + 1) * ET, :])
                    w3l_t = w3tp.tile([ET, HID], F16, tag="w3t")
                    nc.sync.dma_start(w3l_t[:], w3l_p[kt * ET:(kt + 1) * ET, :])
                    xss = xssp.tile([ET, NCORE, BC], F16, tag="xss")
                    nc.vector.tensor_scalar_mul(xss[:, :, :], xc_h[:, :, :, kt], LSI)
                    for half in range(2):
                        js = slice(half * 4, (half + 1) * 4)
                        lh = xc_h[:, js, :, kt]
                        ll = xc_l[:, js, :, kt]
                        l6 = xss[:, half * 4:(half + 1) * 4, :]
                        last = (kt == KT - 1)
                        for n in range(4):
                            ns = slice(n * 512, (n + 1) * 512)
                            p = psY[half * 4 + n]
                            nc.tensor.matmul(p[:], lh, w3h_t[:, ns],
                                             start=False, stop=False)
                            nc.tensor.matmul(p[:], ll, w3h_t[:, ns],
                                             start=False, stop=False)
                            nc.tensor.matmul(p[:], l6, w3l_t[:, ns],
                                             start=False, stop=last)
                for half in range(2):
                    y3e = y3ep.tile([128, HID], F32, tag="y3e")
                    for n in range(4):
                        nc.vector.tensor_copy(
                            y3e[:, n * 512:(n + 1) * 512], psY[half * 4 + n][:])
                    nc.sync.dma_start(ar_in[half * 128:(half + 1) * 128, :],
                                      y3e[:])

            stk3.close()
            nc.gpsimd.collective_compute(
                "AllReduce", mybir.AluOpType.add,
                replica_groups=[list(range(NCORE))],
                ins=[ar_in.opt()], outs=[ar_out.opt()])

            # ================= finale (replicated) =================
            arT = ar_out[:, :].rearrange("b h -> h b")
            NHT = HID // 128  # 16
            stkF = ExitStack()
            fin = stkF.enter_context(tc.tile_pool(name="fin", bufs=1))
            finy = stkF.enter_context(tc.tile_pool(name="finy", bufs=NHT))
            fin2 = stkF.enter_context(tc.tile_pool(name="fin2", bufs=4))
            psf = stkF.enter_context(tc.tile_pool(name="psf", bufs=4, space="PSUM"))
            if True:
                y3h_t, y3l_t, y3s_t, w5h_t, w5l_t = [], [], [], [], []
                for ht in range(NHT):
                    hs = slice(ht * 128, (ht + 1) * 128)
                    b3T = fin2.tile([128, 1], F32, tag="b3T")
                    nc.sync.dma_start(b3T[:], b3_p[hs])
                    yt = fin2.tile([128, B], F32, tag="yt")
                    nc.sync.dma_start(yt[:], arT[hs, :])
                    yr = fin2.tile([128, B], F32, tag="yr")
                    nc.scalar.activation(yr[:], yt[:], RELU, bias=b3T[:])
                    yh = finy.tile([128, B], F16, tag="y3h")
                    nc.vector.tensor_copy(yh[:], yr[:])
                    y3h_t.append(yh)
                    yl32 = fin2.tile([128, B], F32, tag="yl32")
                    nc.vector.tensor_sub(yl32[:], yr[:], yh[:])
                    yl = finy.tile([128, B], F16, tag="y3l")
                    nc.vector.tensor_copy(yl[:], yl32[:])
                    y3l_t.append(yl)
                    ys = finy.tile([128, B], F16, tag="y3s")
                    nc.vector.tensor_scalar_mul(ys[:], yh[:], LSI)
                    y3s_t.append(ys)
                    t = finy.tile([128, A], F16, tag="w5h")
                    nc.sync.dma_start(t[:], w5h_p[hs, :])
                    w5h_t.append(t)
                    t = finy.tile([128, A], F16, tag="w5l")
                    nc.sync.dma_start(t[:], w5l_p[hs, :])
                    w5l_t.append(t)
                ps_z = psf.tile([A, B], F32, tag="psf")
                for ht in range(NHT):
                    nc.tensor.matmul(ps_z[:], w5h_t[ht][:], y3h_t[ht][:],
                                     start=(ht == 0), stop=False)
                    nc.tensor.matmul(ps_z[:], w5h_t[ht][:], y3l_t[ht][:],
                                     start=False, stop=False)
                    nc.tensor.matmul(ps_z[:], w5l_t[ht][:], y3s_t[ht][:],
                                     start=False, stop=(ht == NHT - 1))
                b5T = fin.tile([A, 1], F32, tag="b5T")
                nc.sync.dma_start(b5T[:], b5_p[:])
                zT = fin.tile([A, B], F32, tag="zT")
                nc.scalar.activation(zT[:], ps_z[:], IDENT, bias=b5T[:])
                zh = fin.tile([A, B], F16, tag="zh")
                nc.vector.tensor_copy(zh[:], zT[:])
                zl32 = fin.tile([A, B], F32, tag="zl32")
                nc.vector.tensor_sub(zl32[:], zT[:], zh[:])
                zl = fin.tile([A, B], F16, tag="zl")
                nc.vector.tensor_copy(zl[:], zl32[:])
                zs = fin.tile([A, B], F16, tag="zs")
                nc.vector.tensor_scalar_mul(zs[:], zh[:], LSI)
                la2h = fin.tile([A, B], F16, tag="la2h")
                nc.sync.dma_start(la2h[:], lah_p[:, :])
                la2l = fin.tile([A, B], F16, tag="la2l")
                nc.sync.dma_start(la2l[:], lal_p[:, :])
                la2s = fin.tile([A, B], F16, tag="la2s")
                nc.vector.tensor_scalar_mul(la2s[:], la2h[:], LSI)
                w6 = {}
                for nm, p in (("ah", w6ah_p), ("al", w6al_p),
                              ("bh", w6bh_p), ("bl", w6bl_p)):
                    t = fin.tile([A, A], F16, tag="w6" + nm)
                    nc.sync.dma_start(t[:], p[:, :])
                    w6[nm] = t
                ps_f = psf.tile([A, B], F32, tag="psf")
                nc.tensor.matmul(ps_f[:], w6["ah"][:], zh[:], start=True, stop=False)
                nc.tensor.matmul(ps_f[:], w6["ah"][:], zl[:], start=False, stop=False)
                nc.tensor.matmul(ps_f[:], w6["al"][:], zs[:], start=False, stop=False)
                nc.tensor.matmul(ps_f[:], w6["bh"][:], la2h[:], start=False, stop=False)
                nc.tensor.matmul(ps_f[:], w6["bh"][:], la2l[:], start=False, stop=False)
                nc.tensor.matmul(ps_f[:], w6["bl"][:], la2s[:], start=False, stop=True)
                b6T = fin.tile([A, 1], F32, tag="b6T")
                nc.sync.dma_start(b6T[:], b6_p[:])
                fsT = fin.tile([A, B], F32, tag="fsT")
                nc.scalar.activation(fsT[:], ps_f[:], IDENT, bias=b6T[:])

                for half in range(2):
                    hs = slice(half * 128, (half + 1) * 128)
                    tr_ps = psf.tile([128, A], F32, tag="psf")
                    nc.tensor.transpose(tr_ps[:], fsT[:, hs], id32[:A, :A])
                    fs = fin2.tile([128, A], F32, tag="fs")
                    nc.vector.tensor_copy(fs[:], tr_ps[:])
                    # top-50 mask via iterated max8 + match_replace
                    work = fin2.tile([128, A], F32, tag="work")
                    mx8 = fin2.tile([128, 8], F32, tag="mx8")
                    src = fs
                    for k_on in range(0, NW, 8):
                        kthis = min(8, NW - k_on)
                        nc.vector.max(mx8[:], src[:])
                        if kthis < 8:
                            nc.vector.memset(mx8[:, kthis:], -1e30)
                        nc.vector.match_replace(work[:], in_to_replace=mx8[:],
                                                in_values=src[:],
                                                imm_value=-1e30)
                        src = work
                    mask = fin2.tile([128, A], F32, tag="mask")
                    nc.vector.tensor_sub(mask[:], fs[:], work[:])
                    nc.vector.tensor_scalar_min(mask[:], mask[:], 1.0)
                    # masked scores = fs + (mask - 1) * 1e9, winner-exact
                    mt = fin2.tile([128, A], F32, tag="mt")
                    nc.vector.tensor_scalar(mt[:], mask[:], -1.0, 1e9,
                                            mybir.AluOpType.add, MULT)
                    ms = fin2.tile([128, A], F32, tag="ms")
                    nc.vector.tensor_add(ms[:], fs[:], mt[:])
                    nc.vector.max(mx8[:], ms[:])
                    ngm = fin2.tile([128, 1], F32, tag="ngm")
                    nc.vector.tensor_scalar_mul(ngm[:], mx8[:, 0:1], -1.0)
                    e_f = fin2.tile([128, A], F32, tag="e_f")
                    sm = fin2.tile([128, 1], F32, tag="sm")
                    nc.scalar.activation(e_f[:], ms[:], EXP, bias=ngm[:],
                                         accum_out=sm[:])
                    rcf = fin2.tile([128, 1], F32, tag="rcf")
                    nc.vector.reciprocal(rcf[:], sm[:])
                    p_t = fin2.tile([128, A], F32, tag="p_t")
                    nc.vector.tensor_tensor(p_t[:], e_f[:],
                                            rcf[:].to_broadcast([128, A]), MULT)
                    nc.sync.dma_start(out_p[hs, :], p_t[:])
            stkF.close()

    nc.compile()
    return nc


def _prep_inputs(inputs):
    hist = np.ascontiguousarray(inputs["hist_state"], np.float32)
    la = np.ascontiguousarray(inputs["last_action"], np.float32)
    w3 = np.ascontiguousarray(inputs["w3"], np.float32)

    weh, wel = _split(inputs["w_embed"].T)
    winh = np.ascontiguousarray(inputs["in_proj_w"].T, np.float32).astype(np.float16)
    wouth = np.ascontiguousarray(inputs["out_proj_w"].T, np.float32).astype(np.float16)
    lah, lal = _split0(la.T)
    w3lah, w3lal = _split(w3[:, L * E:].T / 8.0)
    w5h, w5l = _split(inputs["w5"].T)
    w6T = np.ascontiguousarray(inputs["w6"], np.float32).T
    w6ah, w6al = _split(w6T[:A])
    w6bh, w6bl = _split(w6T[A:])

    common = {
        "weh": weh, "wel": wel,
        "be": np.ascontiguousarray(inputs["b_embed"], np.float32),
        "win": winh, "wout": wouth,
        "b3": np.ascontiguousarray(inputs["b3"], np.float32),
        "lah": lah, "lal": lal,
        "w3lah": w3lah, "w3lal": w3lal,
        "w5h": w5h, "w5l": w5l,
        "b5": np.ascontiguousarray(inputs["b5"], np.float32),
        "w6ah": w6ah, "w6al": w6al, "w6bh": w6bh, "w6bl": w6bl,
        "b6": np.ascontiguousarray(inputs["b6"], np.float32),
    }
    # w3 body reshaped so core i's slice is e-chunk [125i:125(i+1)) for all l,
    # rows ordered (l, e_local) to match the device kt-loop over l.
    w3r = w3[:, :L * E].reshape(HID, L, E)
    in_maps = []
    for i in range(NCORE):
        xT = hist[i * BC:(i + 1) * BC].reshape(TOK, E).T
        xh, xl = _split0(xT)
        w3c = np.ascontiguousarray(
            w3r[:, :, i * ET:(i + 1) * ET].transpose(1, 2, 0)).reshape(FLATP, HID)
        w3h, w3l = _split(w3c)
        m = dict(common)
        m.update({"xh": xh, "xl": xl, "w3h": w3h, "w3l": w3l})
        in_maps.append(m)
    return in_maps


def kernel(**inputs):
    global _CACHED
    if _CACHED is None:
        _CACHED = _build()
    in_maps = _prep_inputs(inputs)
    res = run_bass_kernel_spmd(_CACHED, in_maps, list(range(NCORE)))
    return np.ascontiguousarray(res.results[0]["out"], np.float32)


if __name__ == "__main__":
    rng = np.random.default_rng(0)
    print("building...")
    _build()
    print("built ok")
